# revision 21
# baseline (speedup 1.0000x reference)
"""Trainium2 Bass kernel for nn_CascadingSystem (confidence-gated 2-expert blend).

Computation (reference):
    xf = x.reshape(256, 150528)
    t_out = xf @ W1 + b1            # [256, 2]
    f_out = xf @ W2 + b2            # [256, 2]
    conf  = max(softmax(t_out, 1), 1)
    out   = where(conf > 0.95, t_out, 0.7*t_out + 0.3*f_out)

Strategy (memory-bound; reading x dominates; ~410 B/ns effective DMA/core):
  - Shard the feature dim D=150528 across 8 cores (18816 each). Every core
    streams its d-slice of ALL 256 samples once from HBM and computes the
    partial [4, 256] logits (4 = W1c0, W1c1, W2c0, W2c1) on the tensor
    engine, 147 accumulating matmul chunks of K=128.
  - Precision/bandwidth: fp32 matmuls are 4 cyc/row (PE-bound) and fp32
    data is 4 B/elem. Instead decompose on the host
        x = xh(fp16) + xr,   xr8 = fp8_e4m3(xr * 2^12)
        W = wh(fp16) + wl(fp16),  w8 = fp8_e4m3(W * 2^9)
        logits = xh*wh + xh*wl + (xr8*w8) / 2^21
    3 B/elem -> ~36us stream; PE does 2 fp16/fp8 matmuls (1 cyc/row) per
    chunk => ~32us, chasing the DMA roofline. Max logit error ~1.2e-4.
  - DMA structure (learned the hard way): the two HWDGE queues (sync +
    scalar) share a descriptor-dispatch arbiter; sustained ~410 B/ns needs
    BOTH queues active with comparable, mid-size entries finely
    interleaved. Coarse multi-MB entries (>16KB descriptors) make one
    queue hog dispatch AND returned corrupted data on the first NEFF
    execution. So: per-stream chunk DMAs alternate engines (parity), ramp
    1 -> 18 chunks, descriptors <= 9216 B. W goes first as its own DMAs.
  - Self-init + epilogue trimmed vs the original: per-DMA semaphores are
    kept (34), but the output path is single-engine: scalar waits on the
    PE sem, copies both PSUM accumulators to SBUF itself (no vector hop),
    DMAs out. The matmul sem fires at retire ~50ns before the systolic
    drain lands in PSUM; sem propagation + engine wakeup covers it.
  - Host sums the 8 partial tensors and applies the tiny
    bias/softmax/threshold/blend epilogue on [256, 4] floats.
"""

from contextlib import ExitStack

import ml_dtypes
import numpy as np

import concourse.bass as bass
import concourse.mybir as mybir
from concourse.bass_utils import run_bass_kernel_spmd

NCORES = 8
B = 256            # batch (matmul moving dim)
D = 150528         # 3*224*224
DS = D // NCORES   # 18816 features per core
P = 128            # partitions / contraction tile
J = DS // P        # 147 matmul chunks per core
# j-chunks per DMA: small first chunks let the PE start early, tiny last
# chunk lets it finish right after the final bytes land
SIZES = [1, 2, 4, 6, 8, 10, 12, 14, 16, 18, 18, 16, 12, 6, 3, 1]
assert sum(SIZES) == J
STARTS = [sum(SIZES[:i]) for i in range(len(SIZES))]
NDMA = len(SIZES)
W16C = 8 * J       # fp16 weight cols (wh|wl, 4 each, per chunk)
W8C = 4 * J        # fp8 weight cols (4 per chunk)
T16 = W16C + J * B
T8 = W8C + J * B
XS = 2.0 ** 12     # fp8 residual scale
WS = 2.0 ** 9      # fp8 weight scale
THRESHOLD = 0.95

_CACHE = {}


def _build():
    nc = bass.Bass()
    x16_in = nc.declare_dram_parameter("x16", [P, T16], mybir.dt.float16, isOutput=False)
    x8_in = nc.declare_dram_parameter("x8", [P, T8], mybir.dt.float8e4, isOutput=False)
    out = nc.declare_dram_parameter(
        "partial", [8, 2 * B], mybir.dt.float32, isOutput=True
    )

    with ExitStack() as ctx:
        w16 = ctx.enter_context(nc.sbuf_tensor("w16", [P, W16C], mybir.dt.float16))
        w8 = ctx.enter_context(nc.sbuf_tensor("w8", [P, W8C], mybir.dt.float8e4))
        t16 = []
        t8 = []
        for d in range(NDMA):
            t16.append(
                ctx.enter_context(
                    nc.sbuf_tensor(f"t16_{d}", [P, SIZES[d] * B], mybir.dt.float16)
                )
            )
            t8.append(
                ctx.enter_context(
                    nc.sbuf_tensor(f"t8_{d}", [P, SIZES[d] * B], mybir.dt.float8e4)
                )
            )
        out_sb = ctx.enter_context(
            nc.sbuf_tensor("out_sb", [8, 2 * B], mybir.dt.float32)
        )
        acc16 = ctx.enter_context(nc.psum_tensor("acc16", [8, B], mybir.dt.float32))
        acc8 = ctx.enter_context(nc.psum_tensor("acc8", [4, B], mybir.dt.float32))

        sw16 = ctx.enter_context(nc.semaphore("sw16"))
        sw8 = ctx.enter_context(nc.semaphore("sw8"))
        s16 = [ctx.enter_context(nc.semaphore(f"s16_{d}")) for d in range(NDMA)]
        s8 = [ctx.enter_context(nc.semaphore(f"s8_{d}")) for d in range(NDMA)]
        pe_sem = ctx.enter_context(nc.semaphore("pe"))
        dve_sem = ctx.enter_context(nc.semaphore("dve"))
        osem = ctx.enter_context(nc.semaphore("o"))

        def crange(sems):
            nums = sorted(s.num for s in sems)
            assert nums == list(range(nums[0], nums[-1] + 1))
            return range(nums[0], nums[-1] + 1)

        r_s16 = crange(s16)
        r_s8 = crange(s8)

        # per-stream chunk DMAs alternate engines so each HWDGE engine's
        # queue set carries ~equal bytes (a lopsided split runs one stream
        # at half rate and starves the PE)
        def issue_x(eng, parity):
            for d in range(NDMA):
                do16 = (d % 2 == 0) == (parity == "even")
                if do16:
                    c0 = W16C + STARTS[d] * B
                    eng.dma_start(
                        t16[d][:], x16_in[:, c0 : c0 + SIZES[d] * B]
                    ).then_inc(s16[d], 16)
                else:
                    c0 = W8C + STARTS[d] * B
                    eng.dma_start(
                        t8[d][:], x8_in[:, c0 : c0 + SIZES[d] * B]
                    ).then_inc(s8[d], 16)

        block = ctx.enter_context(nc.Block())

        # Self-initialization: semaphores persist across NEFF executions and
        # other programs may leave them nonzero. Each engine clears the sems
        # it increments, then all engines meet at the NRT pseudo-barrier
        # (runtime-implemented, valid even while bass sems are dirty) before
        # any engine WAITS on a sem another engine cleared. All raw emission
        # into the Block's entry basic block — a @block section ends the
        # entry bb with a branch for its engine, so same-engine code after
        # it would be dead (and a cross-engine barrier would deadlock).
        nc.sync.sem_clear(sw16)
        nc.sync.sem_clear(r_s16)
        nc.scalar.sem_clear(sw8)
        nc.scalar.sem_clear(r_s8)
        nc.scalar.sem_clear(osem)
        nc.tensor.sem_clear(pe_sem)
        nc.vector.sem_clear(dve_sem)
        nc._nrt_pseudo_barrier()

        nc.sync.dma_start(w16[:], x16_in[:, 0:W16C]).then_inc(sw16, 16)
        issue_x(nc.sync, "even")
        nc.scalar.dma_start(w8[:], x8_in[:, 0:W8C]).then_inc(sw8, 16)
        issue_x(nc.scalar, "odd")

        @block.tensor
        def _(tensor):
            tensor.wait_ge(sw16, 16)
            tensor.wait_ge(sw8, 16)
            for d in range(NDMA):
                tensor.wait_ge(s16[d], 16)
                tensor.wait_ge(s8[d], 16)
                for jj in range(SIZES[d]):
                    j = STARTS[d] + jj
                    tensor.matmul(
                        acc16[:],
                        w16[:, 8 * j : 8 * j + 8],
                        t16[d][:, jj * B : jj * B + B],
                        start=(j == 0),
                        stop=(j == J - 1),
                    )
                    mm = tensor.matmul(
                        acc8[:],
                        w8[:, 4 * j : 4 * j + 4],
                        t8[d][:, jj * B : jj * B + B],
                        start=(j == 0),
                        stop=(j == J - 1),
                    )
            mm.then_inc(pe_sem, 1)

        @block.vector
        def _(vector):
            # out_sb cols 0:256 = fp16 psum [8, 256]; cols 256:512 rows 0:4
            # = fp8 residual psum [4, 256] (scaled by XS*WS). DVE (not the
            # Activation engine): tensor_copy needs no ACT_TABLE_LOAD, and
            # the cross-engine dve_sem doubles as the ordering fence for the
            # out DMA — engines run relaxed, and a same-engine dma_start
            # executes AHEAD of in-flight copies and ships stale out_sb
            # (seen on hardware).
            vector.wait_ge(pe_sem, 1)
            # small DVE-side delay: the matmul's sem update fires at retire,
            # ~128 PE cycles before the systolic drain lands in PSUM
            vector.tensor_copy(out_sb[0:1, 0:8], w16[0:1, 0:8])
            vector.tensor_copy(out_sb[0:1, 8:16], w16[0:1, 0:8])
            vector.tensor_copy(out_sb[:, 0:B], acc16[:])
            vector.tensor_copy(out_sb[0:4, B : 2 * B], acc8[:]).then_inc(dve_sem, 1)

        @block.scalar
        def _(scalar):
            scalar.wait_ge(pe_sem, 1)
            scalar.wait_ge(dve_sem, 1)
            # no wait on osem: the Block-exit DRAIN on this engine quiesces
            # its HWDGE queue, which covers the out DMA's completion
            scalar.dma_start(out[:], out_sb[:]).then_inc(osem, 16)

    return nc


def _pack(x, W1, W2):
    xf = np.ascontiguousarray(x, dtype=np.float32).reshape(B, D)
    xh = xf.astype(np.float16)
    xr8 = ((xf - xh.astype(np.float32)) * np.float32(XS)).astype(ml_dtypes.float8_e4m3)

    w4 = np.concatenate(
        [np.asarray(W1, np.float32), np.asarray(W2, np.float32)], axis=1
    )  # [D, 4]
    wh = w4.astype(np.float16)
    wl = (w4 - wh.astype(np.float32)).astype(np.float16)
    w8 = (w4 * np.float32(WS)).astype(ml_dtypes.float8_e4m3)

    xw16 = np.empty((NCORES, P, T16), dtype=np.float16)
    # fp16 W part: col 8j + h*4 + c = (wh,wl)[h][k*DS + j*P + p, c]
    wst = np.stack([wh, wl])  # [2, D, 4]
    xw16[:, :, :W16C] = (
        wst.reshape(2, NCORES, J, P, 4)
        .transpose(1, 3, 2, 0, 4)
        .reshape(NCORES, P, W16C)
    )
    # fp16 x part: col W16C + j*B + b = xh[b, k*DS + j*P + p]
    xw16[:, :, W16C:] = (
        xh.reshape(B, NCORES, J, P).transpose(1, 3, 2, 0).reshape(NCORES, P, J * B)
    )

    xw8 = np.empty((NCORES, P, T8), dtype=ml_dtypes.float8_e4m3)
    xw8[:, :, :W8C] = (
        w8.reshape(NCORES, J, P, 4).transpose(0, 2, 1, 3).reshape(NCORES, P, W8C)
    )
    xw8[:, :, W8C:] = (
        xr8.reshape(B, NCORES, J, P).transpose(1, 3, 2, 0).reshape(NCORES, P, J * B)
    )
    return xw16, xw8


def kernel(x, W1, b1, W2, b2, trace=False, trace_cores=None):
    if "nc" not in _CACHE:
        _CACHE["nc"] = _build()
    nc = _CACHE["nc"]

    xw16, xw8 = _pack(x, W1, W2)
    in_maps = [{"x16": xw16[k], "x8": xw8[k]} for k in range(NCORES)]
    kw = {"trace_cores": trace_cores} if trace_cores else {}
    res = run_bass_kernel_spmd(nc, in_maps, list(range(NCORES)), trace=trace, **kw)
    _CACHE["last_results"] = res

    logits4 = np.zeros((4, B), dtype=np.float64)
    for k in range(NCORES):
        r = res.results[k]["partial"]  # [8, 512]
        logits4 += r[0:4, 0:B] + r[4:8, 0:B]
        logits4 += r[0:4, B : 2 * B].astype(np.float64) / (XS * WS)
    logits4 = logits4.astype(np.float32)

    t_out = logits4[0:2].T + np.asarray(b1, np.float32)  # [256, 2]
    f_out = logits4[2:4].T + np.asarray(b2, np.float32)  # [256, 2]
    m = t_out.max(axis=1, keepdims=True)
    e = np.exp(t_out - m)
    conf = (e / e.sum(axis=1, keepdims=True)).max(axis=1)
    blended = 0.7 * t_out + 0.3 * f_out
    out = np.where((conf > THRESHOLD)[:, None], t_out, blended)
    return out.astype(np.float32)


# revision 22
# speedup vs baseline: 1.1367x; 1.1367x over previous
"""Trainium2 Bass kernel for nn_CascadingSystem (confidence-gated 2-expert blend).

Computation (reference):
    xf = x.reshape(256, 150528)
    t_out = xf @ W1 + b1            # [256, 2]
    f_out = xf @ W2 + b2            # [256, 2]
    conf  = max(softmax(t_out, 1), 1)
    out   = where(conf > 0.95, t_out, 0.7*t_out + 0.3*f_out)

Strategy (memory-bound; reading x dominates; ~410 B/ns effective DMA/core):
  - Shard the feature dim D=150528 across 8 cores (18816 each). Every core
    streams its d-slice of ALL 256 samples once from HBM and computes the
    partial [4, 256] logits (4 = W1c0, W1c1, W2c0, W2c1) on the tensor
    engine, 147 accumulating matmul chunks of K=128.
  - Precision/bandwidth: fp32 matmuls are 4 cyc/row (PE-bound) and fp32
    data is 4 B/elem. Instead decompose on the host
        x = xh(fp16) + xr,   xr8 = fp8_e4m3(xr * 2^12)
        W = wh(fp16) + wl(fp16),  w8 = fp8_e4m3(W * 2^9)
        logits = xh*wh + xh*wl + (xr8*w8) / 2^21
    3 B/elem -> ~36us stream; PE does 2 fp16/fp8 matmuls (1 cyc/row) per
    chunk => ~32us, chasing the DMA roofline. Max logit error ~1.2e-4.
  - DMA structure (learned the hard way): the two HWDGE queues (sync +
    scalar) share a descriptor-dispatch arbiter; sustained ~410 B/ns needs
    BOTH queues active with comparable, mid-size entries finely
    interleaved. Coarse multi-MB entries (>16KB descriptors) make one
    queue hog dispatch AND returned corrupted data on the first NEFF
    execution. So: per-stream chunk DMAs alternate engines (parity), ramp
    1 -> 18 chunks, descriptors <= 9216 B. W goes first as its own DMAs.
  - Self-init + epilogue trimmed vs the original: per-DMA semaphores are
    kept (34), but the output path is single-engine: scalar waits on the
    PE sem, copies both PSUM accumulators to SBUF itself (no vector hop),
    DMAs out. The matmul sem fires at retire ~50ns before the systolic
    drain lands in PSUM; sem propagation + engine wakeup covers it.
  - Host sums the 8 partial tensors and applies the tiny
    bias/softmax/threshold/blend epilogue on [256, 4] floats.
"""

from contextlib import ExitStack

import ml_dtypes
import numpy as np

import concourse.bass as bass
import concourse.mybir as mybir
from concourse.bass_utils import run_bass_kernel_spmd

NCORES = 8
B = 256            # batch (matmul moving dim)
D = 150528         # 3*224*224
DS = D // NCORES   # 18816 features per core
P = 128            # partitions / contraction tile
J = DS // P        # 147 matmul chunks per core
# j-chunks per DMA. Small-descriptor entries pay a ~1.3us per-entry
# dispatch floor, which binds when BOTH queues are in small-entry mode at
# once (the tail) — so keep the bulk in 18-chunk entries and end with a
# single small entry per queue: the last bytes land right after the bulk
# and the PE tail is only 2 chunks.
SIZES = [4, 8, 14, 18, 18, 18, 18, 18, 18, 11, 2]
assert sum(SIZES) == J
STARTS = [sum(SIZES[:i]) for i in range(len(SIZES))]
NDMA = len(SIZES)
W16C = 8 * J       # fp16 weight cols (wh|wl, 4 each, per chunk)
W8C = 4 * J        # fp8 weight cols (4 per chunk)
T16 = W16C + J * B
T8 = W8C + J * B
XS = 2.0 ** 12     # fp8 residual scale
WS = 2.0 ** 9      # fp8 weight scale
THRESHOLD = 0.95

_CACHE = {}


def _build():
    nc = bass.Bass()
    x16_in = nc.declare_dram_parameter("x16", [P, T16], mybir.dt.float16, isOutput=False)
    x8_in = nc.declare_dram_parameter("x8", [P, T8], mybir.dt.float8e4, isOutput=False)
    out = nc.declare_dram_parameter(
        "partial", [8, 2 * B], mybir.dt.float32, isOutput=True
    )

    with ExitStack() as ctx:
        w16 = ctx.enter_context(nc.sbuf_tensor("w16", [P, W16C], mybir.dt.float16))
        w8 = ctx.enter_context(nc.sbuf_tensor("w8", [P, W8C], mybir.dt.float8e4))
        t16 = []
        t8 = []
        for d in range(NDMA):
            t16.append(
                ctx.enter_context(
                    nc.sbuf_tensor(f"t16_{d}", [P, SIZES[d] * B], mybir.dt.float16)
                )
            )
            t8.append(
                ctx.enter_context(
                    nc.sbuf_tensor(f"t8_{d}", [P, SIZES[d] * B], mybir.dt.float8e4)
                )
            )
        out_sb = ctx.enter_context(
            nc.sbuf_tensor("out_sb", [8, 2 * B], mybir.dt.float32)
        )
        acc16 = ctx.enter_context(nc.psum_tensor("acc16", [8, B], mybir.dt.float32))
        acc8 = ctx.enter_context(nc.psum_tensor("acc8", [4, B], mybir.dt.float32))

        sw16 = ctx.enter_context(nc.semaphore("sw16"))
        sw8 = ctx.enter_context(nc.semaphore("sw8"))
        s16 = [ctx.enter_context(nc.semaphore(f"s16_{d}")) for d in range(NDMA)]
        s8 = [ctx.enter_context(nc.semaphore(f"s8_{d}")) for d in range(NDMA)]
        pe_sem = ctx.enter_context(nc.semaphore("pe"))
        dve_sem = ctx.enter_context(nc.semaphore("dve"))
        osem = ctx.enter_context(nc.semaphore("o"))

        def crange(sems):
            nums = sorted(s.num for s in sems)
            assert nums == list(range(nums[0], nums[-1] + 1))
            return range(nums[0], nums[-1] + 1)

        r_s16 = crange(s16)
        r_s8 = crange(s8)

        # per-stream chunk DMAs alternate engines so each HWDGE engine's
        # queue set carries ~equal bytes (a lopsided split runs one stream
        # at half rate and starves the PE)
        def issue_x(eng, parity):
            for d in range(NDMA):
                do16 = (d % 2 == 0) == (parity == "even")
                if do16:
                    c0 = W16C + STARTS[d] * B
                    eng.dma_start(
                        t16[d][:], x16_in[:, c0 : c0 + SIZES[d] * B]
                    ).then_inc(s16[d], 16)
                else:
                    c0 = W8C + STARTS[d] * B
                    eng.dma_start(
                        t8[d][:], x8_in[:, c0 : c0 + SIZES[d] * B]
                    ).then_inc(s8[d], 16)

        block = ctx.enter_context(nc.Block())

        # Self-initialization: semaphores persist across NEFF executions and
        # other programs may leave them nonzero. Each engine clears the sems
        # it increments, then all engines meet at the NRT pseudo-barrier
        # (runtime-implemented, valid even while bass sems are dirty) before
        # any engine WAITS on a sem another engine cleared. All raw emission
        # into the Block's entry basic block — a @block section ends the
        # entry bb with a branch for its engine, so same-engine code after
        # it would be dead (and a cross-engine barrier would deadlock).
        nc.sync.sem_clear(sw16)
        nc.sync.sem_clear(r_s16)
        nc.scalar.sem_clear(sw8)
        nc.scalar.sem_clear(r_s8)
        nc.scalar.sem_clear(osem)
        nc.tensor.sem_clear(pe_sem)
        nc.vector.sem_clear(dve_sem)
        nc._nrt_pseudo_barrier()

        nc.sync.dma_start(w16[:], x16_in[:, 0:W16C]).then_inc(sw16, 16)
        issue_x(nc.sync, "even")
        nc.scalar.dma_start(w8[:], x8_in[:, 0:W8C]).then_inc(sw8, 16)
        issue_x(nc.scalar, "odd")

        @block.tensor
        def _(tensor):
            tensor.wait_ge(sw16, 16)
            tensor.wait_ge(sw8, 16)
            for d in range(NDMA):
                tensor.wait_ge(s16[d], 16)
                tensor.wait_ge(s8[d], 16)
                for jj in range(SIZES[d]):
                    j = STARTS[d] + jj
                    tensor.matmul(
                        acc16[:],
                        w16[:, 8 * j : 8 * j + 8],
                        t16[d][:, jj * B : jj * B + B],
                        start=(j == 0),
                        stop=(j == J - 1),
                    )
                    mm = tensor.matmul(
                        acc8[:],
                        w8[:, 4 * j : 4 * j + 4],
                        t8[d][:, jj * B : jj * B + B],
                        start=(j == 0),
                        stop=(j == J - 1),
                    )
            mm.then_inc(pe_sem, 1)

        @block.vector
        def _(vector):
            # out_sb cols 0:256 = fp16 psum [8, 256]; cols 256:512 rows 0:4
            # = fp8 residual psum [4, 256] (scaled by XS*WS). DVE (not the
            # Activation engine): tensor_copy needs no ACT_TABLE_LOAD, and
            # the cross-engine dve_sem doubles as the ordering fence for the
            # out DMA — engines run relaxed, and a same-engine dma_start
            # executes AHEAD of in-flight copies and ships stale out_sb
            # (seen on hardware).
            vector.wait_ge(pe_sem, 1)
            # small DVE-side delay: the matmul's sem update fires at retire,
            # ~128 PE cycles before the systolic drain lands in PSUM
            vector.tensor_copy(out_sb[0:1, 0:8], w16[0:1, 0:8])
            vector.tensor_copy(out_sb[0:1, 8:16], w16[0:1, 0:8])
            vector.tensor_copy(out_sb[:, 0:B], acc16[:])
            vector.tensor_copy(out_sb[0:4, B : 2 * B], acc8[:]).then_inc(dve_sem, 1)

        @block.scalar
        def _(scalar):
            scalar.wait_ge(pe_sem, 1)
            scalar.wait_ge(dve_sem, 1)
            # no wait on osem: the Block-exit DRAIN on this engine quiesces
            # its HWDGE queue, which covers the out DMA's completion
            scalar.dma_start(out[:], out_sb[:]).then_inc(osem, 16)

    return nc


def _pack(x, W1, W2):
    xf = np.ascontiguousarray(x, dtype=np.float32).reshape(B, D)
    xh = xf.astype(np.float16)
    xr8 = ((xf - xh.astype(np.float32)) * np.float32(XS)).astype(ml_dtypes.float8_e4m3)

    w4 = np.concatenate(
        [np.asarray(W1, np.float32), np.asarray(W2, np.float32)], axis=1
    )  # [D, 4]
    wh = w4.astype(np.float16)
    wl = (w4 - wh.astype(np.float32)).astype(np.float16)
    w8 = (w4 * np.float32(WS)).astype(ml_dtypes.float8_e4m3)

    xw16 = np.empty((NCORES, P, T16), dtype=np.float16)
    # fp16 W part: col 8j + h*4 + c = (wh,wl)[h][k*DS + j*P + p, c]
    wst = np.stack([wh, wl])  # [2, D, 4]
    xw16[:, :, :W16C] = (
        wst.reshape(2, NCORES, J, P, 4)
        .transpose(1, 3, 2, 0, 4)
        .reshape(NCORES, P, W16C)
    )
    # fp16 x part: col W16C + j*B + b = xh[b, k*DS + j*P + p]
    xw16[:, :, W16C:] = (
        xh.reshape(B, NCORES, J, P).transpose(1, 3, 2, 0).reshape(NCORES, P, J * B)
    )

    xw8 = np.empty((NCORES, P, T8), dtype=ml_dtypes.float8_e4m3)
    xw8[:, :, :W8C] = (
        w8.reshape(NCORES, J, P, 4).transpose(0, 2, 1, 3).reshape(NCORES, P, W8C)
    )
    xw8[:, :, W8C:] = (
        xr8.reshape(B, NCORES, J, P).transpose(1, 3, 2, 0).reshape(NCORES, P, J * B)
    )
    return xw16, xw8


def kernel(x, W1, b1, W2, b2, trace=False, trace_cores=None):
    if "nc" not in _CACHE:
        _CACHE["nc"] = _build()
    nc = _CACHE["nc"]

    xw16, xw8 = _pack(x, W1, W2)
    in_maps = [{"x16": xw16[k], "x8": xw8[k]} for k in range(NCORES)]
    kw = {"trace_cores": trace_cores} if trace_cores else {}
    res = run_bass_kernel_spmd(nc, in_maps, list(range(NCORES)), trace=trace, **kw)
    _CACHE["last_results"] = res

    logits4 = np.zeros((4, B), dtype=np.float64)
    for k in range(NCORES):
        r = res.results[k]["partial"]  # [8, 512]
        logits4 += r[0:4, 0:B] + r[4:8, 0:B]
        logits4 += r[0:4, B : 2 * B].astype(np.float64) / (XS * WS)
    logits4 = logits4.astype(np.float32)

    t_out = logits4[0:2].T + np.asarray(b1, np.float32)  # [256, 2]
    f_out = logits4[2:4].T + np.asarray(b2, np.float32)  # [256, 2]
    m = t_out.max(axis=1, keepdims=True)
    e = np.exp(t_out - m)
    conf = (e / e.sum(axis=1, keepdims=True)).max(axis=1)
    blended = 0.7 * t_out + 0.3 * f_out
    out = np.where((conf > THRESHOLD)[:, None], t_out, blended)
    return out.astype(np.float32)
